# revision 2
# baseline (speedup 1.0000x reference)
"""DCN layer kernel for Trainium2 (raw Bass), 8-core data parallel. v5.

Computes out = x_0 * (x_l @ w) + b[:, 0] + x_l for
x_l, x_0: [65536, 1024] f32, w, b: [1024, 1] f32.

Sharding: batch dim split evenly across 8 NeuronCores; w/b replicated.

v5 = v1's pipeline verbatim for the steady-state middle (measured at the
HBM roofline: 264.8us/rep marginal = 362 GB/s/core; every layout or
instruction-mix change regressed it - v2/v3/v4 data), plus single-shot
ramp/drain optimization on the two EDGE tiles only:

  - tile 0 and the last tile are loaded as 8 x 1MB block DMAs, each on
    its own dedicated semaphore (exact 16/32 thresholds - no multi-DMA
    counting ambiguity), and computed per 128-row block with the fused
    2-op form (A_k: product+accum dot -> s_k; B_k: o = x0*s_k + xl with
    s_k as a [P,1] per-partition scalar). DVE starts ~5us after launch
    instead of ~24us (after the full 8MB tile-0 load), and the last
    tile's compute trails its own block loads instead of waiting for
    the full tile.
  - the last tile's stores are 8 x 0.5MB block DMAs gated on per-block
    dve_sem increments: the post-compute tail is ~3us instead of ~13us.

Middle tiles: one 8MB interleaved load + 4 batched DVE ops + one 4MB
store per tile, 2-slot ring, chain semaphore after every DVE op
(same-engine RAW: operand reads race ahead of in-flight writes,
verified on HW). All cross-engine thresholds are full-cumulative, so
no partial-completion combination can satisfy a wait early.
"""

from contextlib import ExitStack

import numpy as np

import concourse.bass as bass
from concourse import mybir
from concourse import bass_utils

P = 128  # SBUF partitions
N_CORES = 8
K = 8  # row blocks per tile (8MB x-tile)
XB = 2  # x ring slots

f32 = mybir.dt.float32
MUL = mybir.AluOpType.mult
ADD = mybir.AluOpType.add


def _build(nb, dim, with_b, repeat=1):
    """Per-core program: nb 128-row blocks of width dim, K blocks per tile."""
    assert nb % K == 0
    nt = nb // K
    nit = nt * repeat  # repeat>1 re-runs the pipeline for wall-clock timing
    assert nit >= 2
    nc = bass.Bass("TRN2", target_bir_lowering=False, debug=False,
                   enable_asserts=False)
    xin = nc.dram_tensor("xin", [nb, P, 2, dim], f32, kind="ExternalInput").ap()
    w_rep_d = nc.dram_tensor("w_rep_in", [P, dim], f32, kind="ExternalInput").ap()
    if with_b:
        b_rep_d = nc.dram_tensor("b_rep_in", [P, dim], f32, kind="ExternalInput").ap()
    out = nc.dram_tensor("out", [nb * P, dim], f32, kind="ExternalOutput").ap()

    xin_t = xin.rearrange("(t k) p c d -> t p k c d", k=K)  # [nt, P, K, 2, dim]
    out_t = out.rearrange("(t k p) d -> t p k d", p=P, k=K)  # [nt, P, K, dim]

    n_const = 1 + int(with_b)

    fine_tiles = {0, nit - 1}  # per-block loads + fused per-block compute
    fine_store = {nit - 1}  # per-block stores

    # --- precomputed full-cumulative semaphore thresholds ---
    load_thresh = {}  # t -> threshold on load_sems[sl] (coarse tiles)
    ramp_thresh = {}  # t -> per-block threshold on ramp_sems[k] (fine tiles)
    loads_on_slot = [0] * XB
    ramp_uses = 0
    for t in range(nit):
        sl = t % XB
        if t in fine_tiles:
            ramp_uses += 1
            ramp_thresh[t] = 16 * ramp_uses
        else:
            loads_on_slot[sl] += 1
            load_thresh[t] = 16 * loads_on_slot[sl]
    store_incs = [8 * 16 if t in fine_store else 16 for t in range(nit)]
    store_gate = {}  # t -> threshold on store_sems[sl] before slot reuse
    for t in range(XB, nit):
        store_gate[t] = sum(store_incs[u] for u in range(t - XB, -1, -XB))

    with ExitStack() as ctx:
        e = ctx.enter_context
        xbuf = e(nc.sbuf_tensor([P, XB, K, 2, dim], f32))
        tmp = e(nc.sbuf_tensor([P, K, dim], f32))
        wrep = e(nc.sbuf_tensor([P, dim], f32))
        brep = e(nc.sbuf_tensor("brep", [P, dim], f32)) if with_b else None
        s = e(nc.sbuf_tensor([P, K], f32))
        const_sem = e(nc.semaphore("const_sem"))
        ramp_sems = [e(nc.semaphore(f"ramp_sem{k}")) for k in range(K)]
        load_sems = [e(nc.semaphore(f"load_sem{j}")) for j in range(XB)]
        store_sems = [e(nc.semaphore(f"store_sem{j}")) for j in range(XB)]
        dve_sem = e(nc.semaphore("dve_sem"))
        chain_sem = e(nc.semaphore("chain_sem"))
        block = e(nc.Block())

        @block.sync
        def _(sync):
            sync.dma_start(out=wrep[:, :], in_=w_rep_d[:, :]).then_inc(const_sem, 16)
            if with_b:
                sync.dma_start(out=brep[:, :], in_=b_rep_d[:, :]).then_inc(
                    const_sem, 16
                )
            for t in range(nit):
                sl = t % XB
                if t in store_gate:
                    # slot free only after its previous store (o lives in the
                    # x_0 half of the slot) fully landed in DRAM
                    sync.wait_ge(store_sems[sl], store_gate[t])
                if t in fine_tiles:
                    for k in range(K):
                        sync.dma_start(
                            out=xbuf[:, sl, k, :, :], in_=xin_t[t % nt][:, k, :, :]
                        ).then_inc(ramp_sems[k], 16)
                else:
                    sync.dma_start(
                        out=xbuf[:, sl, :, :, :], in_=xin_t[t % nt]
                    ).then_inc(load_sems[sl], 16)

        # chain_sem count after the final op of block k of the LAST tile;
        # the scalar engine gates that tile's per-block stores on these.
        last_tile_marks = []

        @block.vector
        def _(vector):
            cnt = [0]

            def chain(inst):
                inst.then_inc(chain_sem, 1)
                cnt[0] += 1
                vector.wait_ge(chain_sem, cnt[0])
                return inst

            vector.wait_ge(const_sem, 16 * n_const)
            w_b = wrep[:, None, :].broadcast_to([P, K, dim])
            if with_b:
                b_b = brep[:, None, :].broadcast_to([P, K, dim])
            s_b = s[:, :, None].broadcast_to([P, K, dim])
            for t in range(nit):
                sl = t % XB
                if t in fine_tiles:
                    is_last = t in fine_store
                    for k in range(K):
                        vector.wait_ge(ramp_sems[k], ramp_thresh[t])
                        xl_k = xbuf[:, sl, k, 0, :]
                        x0_k = xbuf[:, sl, k, 1, :]
                        # A_k: tmp_k = xl_k * w, s[k] = sum_d tmp_k
                        chain(nc.vector.scalar_tensor_tensor(
                            out=tmp[:, k, :], in0=xl_k, scalar=1.0,
                            in1=wrep[:, :], op0=MUL, op1=MUL,
                            accum_out=s[:, k : k + 1],
                        ))
                        # B_k: o_k = x0_k * s_k + xl_k
                        last = nc.vector.scalar_tensor_tensor(
                            out=x0_k, in0=x0_k, scalar=s[:, k : k + 1],
                            in1=xl_k, op0=MUL, op1=ADD,
                        )
                        if with_b:
                            chain(last)
                            last = nc.vector.scalar_tensor_tensor(
                                out=x0_k, in0=x0_k, scalar=0.0,
                                in1=brep[:, :], op0=ADD, op1=ADD,
                            )
                        if is_last:
                            # chain inc doubles as the store-gate mark and
                            # (via the wait below) the RAW guard for A_{k+1}
                            last.then_inc(chain_sem, 1)
                            cnt[0] += 1
                            last_tile_marks.append(cnt[0])
                            if k < K - 1:
                                vector.wait_ge(chain_sem, cnt[0])
                        else:
                            if k == K - 1:
                                last.then_inc(dve_sem, 1)
                            else:
                                chain(last)  # A_{k+1} writes s; keep RAW-safe
                else:
                    vector.wait_ge(load_sems[sl], load_thresh[t])
                    xl = xbuf[:, sl, :, 0, :]  # [P, K, dim]
                    x0 = xbuf[:, sl, :, 1, :]  # [P, K, dim]; overwritten by o
                    chain(nc.vector.scalar_tensor_tensor(
                        out=tmp[:, :, :], in0=xl, scalar=1.0, in1=w_b,
                        op0=MUL, op1=MUL,
                    ))
                    chain(nc.vector.tensor_reduce(
                        s[:, :], tmp[:, :, :], axis=mybir.AxisListType.X, op=ADD
                    ))
                    chain(nc.vector.scalar_tensor_tensor(
                        out=tmp[:, :, :], in0=x0, scalar=1.0, in1=s_b,
                        op0=MUL, op1=MUL,
                    ))
                    if with_b:
                        chain(nc.vector.scalar_tensor_tensor(
                            out=x0, in0=tmp[:, :, :], scalar=0.0, in1=xl,
                            op0=ADD, op1=ADD,
                        ))
                        last = nc.vector.scalar_tensor_tensor(
                            out=x0, in0=x0, scalar=0.0, in1=b_b,
                            op0=ADD, op1=ADD,
                        )
                    else:
                        last = nc.vector.scalar_tensor_tensor(
                            out=x0, in0=tmp[:, :, :], scalar=0.0, in1=xl,
                            op0=ADD, op1=ADD,
                        )
                    last.then_inc(dve_sem, 1)

        @block.scalar
        def _(scalar):
            dve_needed = 0
            emitted = [0] * XB
            for t in range(nit):
                sl = t % XB
                if t in fine_store:
                    assert len(last_tile_marks) == K
                    for k in range(K):
                        scalar.wait_ge(chain_sem, last_tile_marks[k])
                        scalar.dma_start(
                            out=out_t[t % nt][:, k, :], in_=xbuf[:, sl, k, 1, :]
                        ).then_inc(store_sems[sl], 16)
                else:
                    dve_needed += 1
                    scalar.wait_ge(dve_sem, dve_needed)
                    scalar.dma_start(
                        out=out_t[t % nt], in_=xbuf[:, sl, :, 1, :]
                    ).then_inc(store_sems[sl], 16)
                emitted[sl] += store_incs[t]
            # drain: all stores landed before program end
            for j in range(XB):
                if emitted[j]:
                    scalar.wait_ge(store_sems[j], emitted[j])

    return nc


_cache = {}


def _get_module(nb, dim, with_b, repeat=1):
    key = (nb, dim, with_b, repeat)
    if key not in _cache:
        _cache[key] = _build(nb, dim, with_b, repeat)
    return _cache[key]


def make_inputs(x_l, x_0, w, b, n_cores=N_CORES):
    """Host-side shard + interleave. Returns (in_maps, with_b, nb, dim)."""
    rows, dim = x_l.shape
    assert rows % (n_cores * P) == 0
    bl = rows // n_cores
    nb = bl // P
    with_b = bool(np.any(b))
    xin = np.stack([x_l, x_0], axis=1)  # [rows, 2, dim]
    w_rep = np.ascontiguousarray(np.broadcast_to(w.reshape(1, dim), (P, dim)))
    if with_b:
        b_rep = np.ascontiguousarray(np.broadcast_to(b.reshape(1, dim), (P, dim)))
    in_maps = []
    for i in range(n_cores):
        m = {
            "xin": xin[i * bl : (i + 1) * bl].reshape(nb, P, 2, dim),
            "w_rep_in": w_rep,
        }
        if with_b:
            m["b_rep_in"] = b_rep
        in_maps.append(m)
    return in_maps, with_b, nb, dim


def run_sharded(x_l, x_0, w, b, trace=False, repeat=1, **kw):
    in_maps, with_b, nb, dim = make_inputs(x_l, x_0, w, b)
    nc = _get_module(nb, dim, with_b, repeat=repeat)
    res = bass_utils.run_bass_kernel_spmd(
        nc, in_maps, core_ids=list(range(N_CORES)), trace=trace, **kw
    )
    out = np.concatenate([res.results[i]["out"] for i in range(N_CORES)], axis=0)
    return out, res


def kernel(x_l, x_0, w, b):
    out, _ = run_sharded(
        np.asarray(x_l), np.asarray(x_0), np.asarray(w), np.asarray(b)
    )
    return out.astype(np.float32, copy=False)
